# revision 27
# baseline (speedup 1.0000x reference)
"""Distributed Trainium2 Bass kernel for the DriftingField problem (V3).

Math (reference):
    targets = [gen; pos]                         # [T, D], T = G + P
    d2[i,j] = |gen_i|^2 + |tgt_j|^2 - 2 gen_i.tgt_j
    dist    = sqrt(d2) / sqrt(D); dist[i,i] = 1e6 (gen block diag)
    K       = exp(-dist / TEMP)                  # [G, T]
    nk      = K / sqrt(max(rs_i * cs_j, 1e-12))
    out     = (nk[:,G:] * s_gen) @ pos - (nk[:,:G] * s_pos) @ gen

Numerical facts exploited:
  * For this data every K entry ~ exp(-28), so rs*cs << 1e-12 and the
    clamp is always active: normalizer == 1e-6 and
        out = 1e12 * [ rg_i * (K[:,G:] @ pos) - rp_i * (K[:,:G] @ gen) ].
    A host guard falls back to exact numpy if the clamp regime is left.
  * d2 concentrates in a +-6 sigma band, so the exponent
    E(d2) = -s*sqrt(d2) admits a K^2*prob-weighted quadratic fit
        E ~= lam^2*(d2 - V)^2 + w          (V beyond the data range)
    which turns V2's 3-op Ln->Exp->Exp ACT chain into a TWO-op
    Square->Exp chain:  q = (scale*ps + bias)^2 ; K = exp(q + w),
    scale = -2*lam, bias = lam*(b2 + C0 - V) per target row.  The fit
    is computed at runtime from a d2 subsample of the actual inputs; a
    per-call range check falls back to exact numpy on drift.  Measured
    end-to-end rel_norm 0.0182 (fp8 input rounding dominates, as in V2).

V3 layout/schedule (per core, 512 gen rows), HW ~250 us (V2: 291 us):
  * d2 assembly is ENTIRELY on the PE: per j-tile one PSUM accumulation
    group of 6 matmuls -- 4 fp8e4m3 DoubleRow cross-term matmuls first
    (gated only on the tTb/genT streams), then [diag fill: bf16 ident x
    per-core sliding-window mask of -V/2, landing d2_diag at the FLAT
    vertex V so K_diag = e^w ~ 0] + [a2-fold: K=128-padded bf16 matmul,
    hi/lo rows of (a2-1024)/8 x const columns -4, -4/256].  No DVE or
    gpsimd op touches PSUM (gpsimd cannot), and no a2 broadcast chain.
    NOTE: a 2-partition stationary hard-crashed the device (status 101)
    -- tiny-K stationaries must be padded to K=128.
  * b2 per tile: square on Pool (2 of 3 tiles) / DVE (1 of 3), free-axis
    reduce + affine-to-bias on DVE.  (tensor_tensor_reduce crashes HW;
    split ops are reliable.)
  * a2 at startup: ACT Square accum of f32 gen rows; hi/lo bf16 encode
    in column layout; ONE PE transpose ([128,8] -> [8,128] via identity)
    + 2 partition-folding DMAs land the rows on Xpair partitions 0/1.
  * Phase 1 also accumulates ic0's dh1 half (md1, the PSUM bank freed
    by ps_pool 4->3), sharing kt0's stationary with its dh0 matmul:
    PSUM = 3 ps + 4 mout + 1 md1 = 8 banks.
  * Phase 2: dh1 matmuls for ic 1..3 + row sums rg/rp via rgt matmuls
    sharing the m2 stationaries (one bank per accumulation group --
    shared-bank groups corrupt on HW).  POS half first so alpha=1e12*rp
    is ready mid-phase.  vt tiles prefetched 6-8 deep on ALTERNATING
    scalar/sync hwdge queues (single-queue vt loads stall PE ~40us:
    the sync queue also carries all semaphore waits).  Gen-half dh1
    stays in PSUM for the tail combine (no eviction copies); combine
    scratch is an 8-buf pool (a 2-buf ring serialized the tail on the
    store->reuse dependency).
"""

import numpy as np
from contextlib import ExitStack

import concourse.bacc as bacc
import concourse.hw_specs as hw_specs
import concourse.mybir as mybir
import concourse.tile as tile
from concourse.bass_utils import run_bass_kernel_spmd

F32 = mybir.dt.float32
BF16 = mybir.dt.bfloat16
AF = mybir.ActivationFunctionType
ALU = mybir.AluOpType

# Route every activation function this kernel uses (Exp/Square/Copy) to
# ONE table set so the load-insertion pass hoists a single
# ACT_TABLE_LOAD instead of thrashing per-function table loads.
_PREF_ACT_SET = "natural_log_exp_and_others"
_ROUTED_TABLES = {}


def _routed_activation_tables(arch):
    if arch not in _ROUTED_TABLES:
        base = hw_specs.get_activation_tables(arch)
        strip = {AF.Ln, AF.Exp, AF.Square, AF.Copy, AF.Identity}
        _ROUTED_TABLES[arch] = {
            name: (set(funcs) if name == _PREF_ACT_SET else set(funcs) - strip)
            for name, funcs in base.items()
        }
    return _ROUTED_TABLES[arch]


bacc.get_activation_tables = _routed_activation_tables

NCORES = 8
TEMP = 0.05
CLAMP = 1.0e-12          # reference: max(rs*cs, 1e-12)
INV_NORM2 = 1.0 / CLAMP  # 1e12, the (1/normalizer)^2 when clamped
C0 = 1024.0              # a2 encode offset: r = (a2 - C0)/8

TRACE = False
LAST_RESULT = None


def build_nc(G, P, D, lam, V, wfit):
    T = G + P
    RPC = G // NCORES          # gen rows per core
    NJ = T // 128              # j-subtiles (target rows)
    NI = RPC // 128            # i-chunks (this core's gen rows)
    ND = D // 128              # d-chunks (feature dim)
    WFILL = T + RPC            # sliding-window mask width
    NJG = G // 128             # j-subtiles in the gen block
    NG = 8                     # j-subtiles per stationary-group DMA
    LAG = 3                    # output-matmul pipeline offset (j-tiles)
    DH = 512                   # output free-dim half width
    SCALE_ACT = -2.0 * lam     # Square: q = (SCALE_ACT*ps + bias)^2
    BIAS_C = lam * (C0 - V)    # bias = lam*b2 + BIAS_C (per target row)

    nc = bacc.Bacc(trn_type="TRN2", num_devices=NCORES)

    F8 = mybir.dt.float8e4
    gen_rows = nc.dram_tensor("gen_rows", [RPC, D], F32, kind="ExternalInput")
    genT_f8 = nc.dram_tensor("genT_f8", [D, RPC], F8, kind="ExternalInput")
    targets_bf = nc.dram_tensor("targets_bf", [T, D], BF16, kind="ExternalInput")
    targets_T_f8 = nc.dram_tensor("targets_T_f8", [D, T], F8,
                                  kind="ExternalInput")
    maskw_bf = nc.dram_tensor("maskw_bf", [128, WFILL], BF16,
                              kind="ExternalInput")
    ident_in = nc.dram_tensor("ident_in", [128, 128], BF16,
                              kind="ExternalInput")
    a2c_in = nc.dram_tensor("a2c_in", [128, 128], BF16, kind="ExternalInput")
    out = nc.dram_tensor("out", [RPC, D], F32, kind="ExternalOutput")
    cs_part = nc.dram_tensor("cs_part", [128, NJ], F32, kind="ExternalOutput")
    rs_out = nc.dram_tensor("rs_out", [128, NI], F32, kind="ExternalOutput")
    rs_stage = nc.dram_tensor("rs_stage", [2, RPC], F32)

    with tile.TileContext(nc) as tc, ExitStack() as ctx:
        const = ctx.enter_context(tc.tile_pool(name="const", bufs=1))
        work = ctx.enter_context(tc.tile_pool(name="work", bufs=2))
        cmbp = ctx.enter_context(tc.tile_pool(name="cmbp", bufs=8))

        ones_bf = const.tile([128, 1], BF16, tag="ones_bf")
        nc.vector.memset(ones_bf, 1.0)
        wbias = const.tile([128, 1], F32, tag="wbias")
        nc.vector.memset(wbias, wfit)

        bias_sb = const.tile([128, NJ], F32, tag="bias_sb")
        cs_sb = const.tile([128, NJ], F32, tag="cs_sb")
        genT = const.tile([128, ND, RPC], F8, tag="genT")
        maskw = const.tile([128, WFILL], BF16, tag="maskw")
        ident = const.tile([128, 128], BF16, tag="ident")
        a2h_col = const.tile([128, NI], F32, tag="a2h_col")
        Xpair = const.tile([128, RPC], BF16, tag="Xpair")
        a2c = const.tile([128, 128], BF16, tag="a2c")
        sgen = const.tile([128, NI, D], F32, tag="sgen")
        spos = const.tile([128, NI, D], F32, tag="spos")
        alpha = const.tile([128, NI], F32, tag="alpha")
        beta = const.tile([128, NI], F32, tag="beta")
        beta_t = const.tile([128, NI], F32, tag="beta_t")
        alpha_t = const.tile([128, NI], F32, tag="alpha_t")
        ab_t = {0: beta_t, 1: alpha_t}

        # a2-fold stationary (host const): ps += -4*hi - (4/256)*lo
        # == -(a2-C0)/2.  K padded to 128 (rows 2.. are zero): tiny-K
        # stationaries are an untested HW corner, full-K is the safe shape.
        nc.scalar.dma_start(out=a2c, in_=a2c_in[:, :])
        nc.vector.memset(Xpair, 0.0)

        # ---- pools for the pipelined main loop ----
        kt_pool = ctx.enter_context(tc.tile_pool(name="kt_pool", bufs=NJ))
        tTb_pool = ctx.enter_context(tc.tile_pool(name="tTb_pool", bufs=2))
        tbf_pool = ctx.enter_context(tc.tile_pool(name="tbf_pool", bufs=8))

        # Startup order on the SP queue: gen_rows first (feeds the a2
        # chain, which gates every Square bias), then matmul operands,
        # then the tbf stream.
        def load_square_a2(ic, eng=None):
            gci = work.tile([128, D], F32, tag="f32big")
            (eng or nc.sync).dma_start(
                out=gci, in_=gen_rows[ic * 128:(ic + 1) * 128, :])
            sq_scr = work.tile([128, D], BF16, tag="sqscr")
            nc.scalar.activation(sq_scr, gci, AF.Square,
                                 accum_out=a2h_col[:, ic:ic + 1])

        nc.scalar.dma_start(out=ident, in_=ident_in[:, :])
        load_square_a2(0)
        load_square_a2(1)
        load_square_a2(2)
        load_square_a2(3)
        nc.sync.dma_start(
            out=genT,
            in_=genT_f8[:, :].rearrange("(c p) i -> p c i", p=128))
        tTbs = {}
        tbfs = {}

        def issue_tTb(g):
            tTbs[g] = tTb_pool.tile([128, ND, NG * 128], F8, tag="tTb",
                                    name=f"tTb{g % 2}")
            j0 = g * NG * 128
            nc.sync.dma_start(
                out=tTbs[g],
                in_=targets_T_f8[:, j0:j0 + NG * 128].rearrange(
                    "(c p) j -> p c j", p=128))

        def issue_tbf(js):
            tbfs[js] = tbf_pool.tile([128, D], BF16, tag="tbf",
                                     name=f"tbf{js % 8}")
            nc.sync.dma_start(out=tbfs[js],
                              in_=targets_bf[js * 128:(js + 1) * 128, :])

        issue_tTb(0)
        nc.sync.dma_start(out=maskw, in_=maskw_bf[:, :])
        for js in range(4):
            issue_tbf(js)

        # ---- a2 -> hi/lo bf16 rows on the 2 Xpair partitions ----
        # r = (a2 - C0)/8 in bf16; hi = bf16(r); lo = 256*(r - hi).
        rcol = const.tile([128, NI], F32, tag="rcol")
        nc.vector.tensor_scalar(out=rcol, in0=a2h_col, scalar1=0.125,
                                scalar2=-128.0, op0=ALU.mult, op1=ALU.add)
        hi_col = const.tile([128, NI], BF16, tag="hi_col")
        nc.vector.tensor_copy(hi_col, rcol)
        dcol = const.tile([128, NI], F32, tag="dcol")
        nc.vector.tensor_tensor(out=dcol, in0=rcol, in1=hi_col,
                                op=ALU.subtract)
        hl_bf = const.tile([128, 2 * NI], BF16, tag="hl_bf")
        nc.vector.tensor_copy(hl_bf[:, 0:NI], hi_col)
        nc.vector.tensor_scalar(out=hl_bf[:, NI:2 * NI], in0=dcol,
                                scalar1=256.0, scalar2=None, op0=ALU.mult)
        prep_stack = ExitStack()
        prep_ps = prep_stack.enter_context(
            tc.tile_pool(name="prep_ps", bufs=1, space="PSUM"))
        rT = prep_ps.tile([2 * NI, 128], BF16, tag="rT")
        nc.tensor.transpose(rT, hl_bf, ident)
        rT_sb = const.tile([2 * NI, 128], BF16, tag="rT_sb")
        nc.scalar.copy(rT_sb, rT)
        # fold partitions 0..3 -> Xpair row 0 (hi), 4..7 -> row 1 (lo)
        nc.scalar.dma_start(out=Xpair[0:1, :], in_=rT_sb[0:NI, :])
        nc.scalar.dma_start(out=Xpair[1:2, :], in_=rT_sb[NI:2 * NI, :])
        prep_stack.close()

        vt_pool = ctx.enter_context(tc.tile_pool(name="vt_pool", bufs=8))
        vts = {}

        def issue_vt(j):
            vts[j] = vt_pool.tile([128, DH], BF16, tag="vt",
                                  name=f"vt{j % 8}")
            (nc.scalar if j % 2 == 0 else nc.sync).dma_start(
                out=vts[j], in_=targets_bf[j * 128:(j + 1) * 128, DH:D])

        ph1 = ExitStack()
        ps_pool = ph1.enter_context(tc.tile_pool(name="ps_pool", bufs=3,
                                                 space="PSUM"))
        mout_pool = ph1.enter_context(tc.tile_pool(name="mout_pool", bufs=1,
                                                   space="PSUM"))
        # the bank freed by ps_pool 4->3 accumulates ic0's dh1 half in
        # phase 1 (tbf holds the full 1024-wide rows anyway), moving a
        # quarter of phase 2's matmuls into phase-1 PE idle time
        md1_pool = ph1.enter_context(tc.tile_pool(name="md1_pool", bufs=1,
                                                  space="PSUM"))

        kts = {}
        mout = {}
        md1 = {}

        def emit_out_dh0(j):
            half = 0 if j < NJG else 1
            j0 = 0 if half == 0 else NJG
            j1 = NJG - 1 if half == 0 else NJ - 1
            if j == j0:
                mout[half] = mout_pool.tile([128, NI, DH], F32, tag="mout",
                                            name=f"mout{half}")
            start = j == j0
            stop = j == j1
            if start:
                md1[half] = md1_pool.tile([128, DH], F32, tag="md1",
                                          name=f"md1_{half}")
            for ic in range(NI):
                lhs = kts[j][:, ic * 128:(ic + 1) * 128]
                nc.tensor.matmul(mout[half][:, ic, :], lhsT=lhs,
                                 rhs=tbfs[j][:, 0:DH],
                                 start=start, stop=stop)
                if ic == 0:  # md1 shares the kt0 stationary
                    nc.tensor.matmul(md1[half], lhsT=lhs,
                                     rhs=tbfs[j][:, DH:D],
                                     start=start, stop=stop)
            if stop:
                dst = sgen if half == 0 else spos
                for ic in range(NI):
                    nc.scalar.copy(dst[:, ic, 0:DH], mout[half][:, ic, :])
                nc.scalar.copy(dst[:, 0, DH:D], md1[half])

        # ---- main loop over target j-tiles ----
        for g in range(NJ // NG):
            if g not in tTbs:
                issue_tTb(g)
            tTb = tTbs[g]
            if g + 1 < NJ // NG:
                issue_tTb(g + 1)
            for k in range(NG):
                js = g * NG + k
                pf = js + 4
                if pf < NJ and pf not in tbfs:
                    issue_tbf(pf)
                tbf = tbfs[js]
                # b2: square on Pool (2 of 3) / DVE (1 of 3), reduce +
                # affine on DVE -- Pool at 1.9us/js was the ph1 pacer
                b2_scr = work.tile([128, D], BF16, tag="sqscr")
                sq_eng = nc.vector if js % 3 == 2 else nc.gpsimd
                sq_eng.tensor_tensor(out=b2_scr, in0=tbf, in1=tbf,
                                     op=ALU.mult)
                nc.vector.tensor_reduce(bias_sb[:, js:js + 1], b2_scr,
                                        axis=mybir.AxisListType.X,
                                        op=ALU.add)
                nc.vector.tensor_scalar(out=bias_sb[:, js:js + 1],
                                        in0=bias_sb[:, js:js + 1],
                                        scalar1=lam, scalar2=BIAS_C,
                                        op0=ALU.mult, op1=ALU.add)
                # d2 assembly: ONE PSUM accumulation group, all on PE.
                # Cross-term DRs FIRST (they only need tTb+genT, ready
                # ~10us) -- a2/diag last so the startup a2 chain never
                # gates the group's first matmul.
                ps = ps_pool.tile([128, RPC], F32, tag="ps", name=f"ps{js % 4}")
                for dp in range(ND // 2):
                    nc.tensor.matmul(
                        ps,
                        lhsT=tTb[:, 2 * dp:2 * dp + 2, k * 128:(k + 1) * 128],
                        rhs=genT[:, 2 * dp:2 * dp + 2, :],
                        start=(dp == 0), stop=False,
                        perf_mode=mybir.MatmulPerfMode.DoubleRow)
                off = T - js * 128
                nc.tensor.matmul(ps, lhsT=ident,
                                 rhs=maskw[:, off:off + RPC],
                                 start=False, stop=False)
                nc.tensor.matmul(ps, lhsT=a2c, rhs=Xpair,
                                 start=False, stop=True)
                # K = exp(lam^2*(d2-V)^2 + w): Square -> Exp (one table set)
                q = work.tile([128, RPC], F32, tag="qsq")
                nc.scalar.activation(q, ps, AF.Square, scale=SCALE_ACT,
                                     bias=bias_sb[:, js:js + 1])
                kt = kt_pool.tile([128, RPC], BF16, tag="kt", name=f"kt{js}")
                kts[js] = kt
                nc.scalar.activation(kt, q, AF.Exp, bias=wbias,
                                     accum_out=cs_sb[:, js:js + 1])
                if js >= LAG:
                    emit_out_dh0(js - LAG)
                if js >= NJ - 8:
                    issue_vt(NJG + (js - (NJ - 8)))  # ph2 pos half prefetch
        for j in range(NJ - LAG, NJ):
            emit_out_dh0(j)

        nc.scalar.dma_start(out=cs_part[:, :], in_=cs_sb)
        ph1.close()  # release phase-1 PSUM pools before phase 2's

        # ---- phase 2: dh1 output matmuls + row sums (rgt shares the
        # m2 stationaries; one PSUM bank per accumulation group).
        # POS half first so alpha=1e12*rp is ready mid-phase; vt loads
        # alternate scalar/sync hwdge queues.
        t2d0 = const.tile([128, NI, DH], F32, tag="t2d0")
        m2_pool = ctx.enter_context(tc.tile_pool(name="m2_pool", bufs=1,
                                                 space="PSUM"))
        rg_pool = ctx.enter_context(tc.tile_pool(name="rg_pool", bufs=NI,
                                                 space="PSUM"))
        m2 = {}
        rgt = {}
        for j in list(range(NJG, NJ)) + list(range(NJG)):
            half = 0 if j < NJG else 1
            j0 = 0 if half == 0 else NJG
            j1 = NJG - 1 if half == 0 else NJ - 1
            if j not in vts:
                issue_vt(j)
            pf = j + 6
            if NJG <= pf < NJ and pf not in vts:
                issue_vt(pf)
            elif pf >= NJ and (pf - NJ) < NJG and (pf - NJ) not in vts:
                issue_vt(pf - NJ)
            vt = vts[j]
            if j == j0:
                m2[half] = m2_pool.tile([128, NI - 1, DH], F32, tag="m2",
                                        name=f"m2{half}")
                for ic in range(NI):
                    rgt[(half, ic)] = rg_pool.tile([128, 1], F32, tag="rgt",
                                                   name=f"rg{half}_{ic}")
            for ic in range(NI):
                lhs = kts[j][:, ic * 128:(ic + 1) * 128]
                if ic > 0:
                    nc.tensor.matmul(m2[half][:, ic - 1, :], lhsT=lhs,
                                     rhs=vt, start=(j == j0), stop=(j == j1))
                nc.tensor.matmul(rgt[(half, ic)], lhsT=lhs, rhs=ones_bf,
                                 start=(j == j0), stop=(j == j1))
            if j == j1:
                ab = beta if half == 0 else alpha
                for ic in range(NI):
                    nc.vector.tensor_scalar_mul(ab[:, ic:ic + 1],
                                                rgt[(half, ic)], INV_NORM2)
                if half == 1:
                    # pos half done mid-phase: spos-dh1 copies hide here
                    for ic in range(1, NI):
                        nc.scalar.copy(spos[:, ic, DH:D],
                                       m2[half][:, ic - 1, :])
                    for ic in range(NI):
                        nc.vector.tensor_scalar_mul(t2d0[:, ic, :],
                                                    sgen[:, ic, 0:DH],
                                                    alpha[:, ic:ic + 1])
        # dh0 combines depend only on beta (spos-dh0 + t2d0 ready):
        for ic in range(NI):
            t1 = cmbp.tile([128, DH], F32, tag="cmb")
            nc.vector.scalar_tensor_tensor(
                out=t1, in0=spos[:, ic, 0:DH], scalar=beta[:, ic:ic + 1],
                in1=t2d0[:, ic, :], op0=ALU.mult, op1=ALU.subtract)
            nc.scalar.dma_start(out=out[ic * 128:(ic + 1) * 128, 0:DH],
                                in_=t1)
        rs_sb = const.tile([128, NI], F32, tag="rs_sb")
        nc.vector.tensor_add(rs_sb, alpha, beta)
        nc.vector.tensor_scalar_mul(rs_sb, rs_sb, 1.0 / INV_NORM2)
        nc.scalar.dma_start(out=rs_out[:, :], in_=rs_sb)
        for ic in range(NI):
            # dh1: ic0's gen half lives in sgen (phase-1 accum); ic>0
            # still in PSUM (m2[0]) -- no tail copy
            src_dh1 = sgen[:, 0, DH:D] if ic == 0 else m2[0][:, ic - 1, :]
            t2 = cmbp.tile([128, DH], F32, tag="cmb")
            nc.vector.tensor_scalar_mul(t2, src_dh1,
                                        alpha[:, ic:ic + 1])
            t3 = cmbp.tile([128, DH], F32, tag="cmb")
            nc.vector.scalar_tensor_tensor(
                out=t3, in0=spos[:, ic, DH:D], scalar=beta[:, ic:ic + 1],
                in1=t2, op0=ALU.mult, op1=ALU.subtract)
            nc.scalar.dma_start(out=out[ic * 128:(ic + 1) * 128, DH:D],
                                in_=t3)

    nc.compile()
    return nc


def make_in_maps(gen, pos, G, P, D, V):
    import ml_dtypes
    T = G + P
    RPC = G // NCORES
    WFILL = T + RPC
    targets = np.concatenate([gen, pos], axis=0).astype(np.float32)
    targets_bf = np.ascontiguousarray(targets.astype(ml_dtypes.bfloat16))
    targets_t_f8 = np.ascontiguousarray(
        targets.T.astype(ml_dtypes.float8_e4m3))
    ident = np.eye(128, dtype=ml_dtypes.bfloat16)
    a2c = np.zeros((128, 128), ml_dtypes.bfloat16)
    a2c[0, :] = ml_dtypes.bfloat16(-4.0)
    a2c[1, :] = ml_dtypes.bfloat16(-0.015625)
    in_maps = []
    p = np.arange(128)
    for c in range(NCORES):
        # diag fill rides a matmul: ident x maskw window adds -V/2 to the
        # cross-term ps (d2 = -2*ps + ...), landing d2_diag at the vertex V
        mask = np.zeros((128, WFILL), ml_dtypes.bfloat16)
        q = T + p - c * RPC
        mask[p, q] = ml_dtypes.bfloat16(-V / 2.0)
        gen_c = np.ascontiguousarray(gen[c * RPC:(c + 1) * RPC]).astype(np.float32)
        genT_f8 = np.ascontiguousarray(gen_c.T.astype(ml_dtypes.float8_e4m3))
        in_maps.append({
            "gen_rows": gen_c,
            "genT_f8": genT_f8,
            "targets_bf": targets_bf,
            "targets_T_f8": targets_t_f8,
            "maskw_bf": mask,
            "ident_in": ident,
            "a2c_in": a2c,
        })
    return in_maps


def _exact_numpy_reference(gen, pos):
    """Bit-faithful (float64) fallback for inputs outside the fitted regime."""
    G, D = gen.shape
    gen64 = gen.astype(np.float64)
    pos64 = pos.astype(np.float64)
    tgt = np.concatenate([gen64, pos64], 0)
    d2 = (gen64 * gen64).sum(-1)[:, None] + (tgt * tgt).sum(-1)[None, :] \
        - 2.0 * gen64 @ tgt.T
    dist = np.sqrt(np.maximum(d2, 0.0))
    if D > 10:
        dist = dist / np.sqrt(D)
    idx = np.arange(G)
    dist[idx, idx] = 1e6
    k = np.exp(-dist / TEMP)
    rs = k.sum(-1, keepdims=True)
    cs = k.sum(-2, keepdims=True)
    nk = k / np.sqrt(np.maximum(rs * cs, CLAMP))
    pos_c = nk[:, G:] * nk[:, :G].sum(-1, keepdims=True)
    neg_c = nk[:, :G] * nk[:, G:].sum(-1, keepdims=True)
    return (pos_c @ pos64 - neg_c @ gen64).astype(np.float32)


def _sample_d2(gen, pos, ng, nt, seed=0):
    """Deterministic strided d2 subsample (no self-pairs kept)."""
    tgt = np.concatenate([gen, pos], 0)
    gs = gen[:: max(1, len(gen) // ng)][:ng].astype(np.float64)
    ts = tgt[:: max(1, len(tgt) // nt)][:nt].astype(np.float64)
    d2 = (gs * gs).sum(-1)[:, None] + (ts * ts).sum(-1)[None, :] \
        - 2.0 * gs @ ts.T
    v = d2.ravel()
    return v[v > 1e-6]


def _fit_quadratic(gen, pos, D):
    """K^2*prob-weighted LSQ fit of E(d2) = -s*sqrt(d2) by a quadratic
    lam^2*(d2-V)^2 + w.  Returns (lam, V, w, lo, hi) or None if the fit
    is unusable (then the caller falls back to exact numpy)."""
    v = _sample_d2(gen, pos, 512, 512)
    if v.size < 1000 or v.min() <= 0:
        return None
    lo, hi = float(v.min()), float(v.max())
    span = hi - lo
    lo_f, hi_f = lo - 0.15 * span, hi + 0.08 * span   # unsampled-tail margin
    if lo_f <= 0:
        return None
    xs = np.linspace(lo_f, hi_f, 4001)
    histw, edges = np.histogram(v, bins=200, range=(lo_f, hi_f), density=True)
    centers = 0.5 * (edges[:-1] + edges[1:])
    wdist = np.interp(xs, centers, histw)
    wdist = wdist + 0.02 * wdist.max()                 # keep tails in the fit
    s = 1.0 / (TEMP * np.sqrt(D))
    E = -s * np.sqrt(xs)
    K = np.exp(E - E.max())
    w = wdist * K ** 2
    A = np.vstack([xs ** 2, xs, np.ones_like(xs)]).T
    W = np.sqrt(w)
    coef, *_ = np.linalg.lstsq(A * W[:, None], E * W, rcond=None)
    a, b, c = (float(x) for x in coef)
    if not (a > 0):
        return None
    V = -b / (2 * a)
    wv = c - b * b / (4 * a)
    if V < hi_f:            # vertex must lie beyond the data range
        return None
    return dict(lam=float(np.sqrt(a)), V=V, w=wv, lo=lo_f, hi=hi_f)


_NC_CACHE = {}


def kernel(gen_features, pos_features):
    global LAST_RESULT
    gen = np.asarray(gen_features, dtype=np.float32)
    pos = np.asarray(pos_features, dtype=np.float32)
    G, D = gen.shape
    P = pos.shape[0]

    key = (G, P, D)
    if key not in _NC_CACHE:
        fit = _fit_quadratic(gen, pos, D)
        if fit is None:
            return _exact_numpy_reference(gen, pos)
        nc = build_nc(G, P, D, fit["lam"], fit["V"], fit["w"])
        _NC_CACHE[key] = (nc, fit)
    nc, fit = _NC_CACHE[key]

    # drift guard: quick d2 subsample must stay inside the fitted range
    vq = _sample_d2(gen, pos, 64, 256)
    if vq.size == 0 or vq.min() < fit["lo"] or vq.max() > fit["hi"]:
        return _exact_numpy_reference(gen, pos)

    in_maps = make_in_maps(gen, pos, G, P, D, fit["V"])
    res = run_bass_kernel_spmd(nc, in_maps, core_ids=list(range(NCORES)),
                               trace=TRACE)
    LAST_RESULT = res
    out = np.concatenate([res.results[c]["out"] for c in range(NCORES)], axis=0)

    # Clamp-regime guard: the device kernel assumes rs_i*cs_j <= 1e-12
    # everywhere (always true for this problem's data). Verify from the
    # device's own row/column sums; fall back to exact evaluation if not.
    cs_glob = sum(res.results[c]["cs_part"] for c in range(NCORES))
    rs_max = max(float(res.results[c]["rs_out"].max()) for c in range(NCORES))
    cs_max = float(cs_glob.max())
    if (not np.isfinite(out).all() or not np.isfinite(rs_max)
            or not np.isfinite(cs_max) or rs_max * cs_max > 0.25 * CLAMP):
        return _exact_numpy_reference(gen, pos)
    return out.astype(np.float32)
